# revision 29
# baseline (speedup 1.0000x reference)
"""TRN2 Bass kernel for nn_Attention_20444044329649.

GroupNorm(32) -> qkv dense -> single-head spatial attention (1024 pos) ->
out dense -> residual.  B=32 examples sharded 4-per-core across 8 cores;
params replicated.

v3 — v2's algebraic folds plus PE-FIFO discipline:

  * scores:  S*isq = Z M' Z^T with M' = isq*Wq Wk^T host-precomputed;
    device computes G^T = M'^T Z^T only (no K projection).  q/k biases:
    per-query term cancels in softmax; per-key term u_j applied as a
    per-partition exp bias (only emitted when b_qkv != 0).
  * out-proj fold: Wv' = Wv W_out, b' = bv W_out + b_out; O computed in
    NATURAL layout via lhsT=ET chunks, rhs=V'.
  * softmax denominators: N=1 matmuls interleaved with the O matmuls.
  * big matmul operands bf16; accumulation fp32 in PSUM; residual fp32.
  * GroupNorm stats batched to exactly TWO tiny PE matmuls per example
    (pool + expand across all 4 channel chunks at once), emitted inside
    the PREVIOUS example's O phase where their DVE inputs are already
    complete — the PE strict-FIFO queue never waits on the serial DVE
    stats chain.
  * next example's x transposes interleaved into the ST j-loop: no-dep
    PE work that keeps the HAM activity window busy (transpose-mode gaps
    plus boundary stalls previously re-throttled the PE to 1.2 GHz for
    ~10 us every example).
  * zt/v/et double-buffered so cross-example WAR hazards never
    serialize; example-0 input DMA spread over 4 queues.
"""

import numpy as np

import concourse.bass as bass
import concourse.mybir as mybir
import concourse.tile as tile
from concourse import bacc
from concourse.bass_utils import run_bass_kernel_spmd
from concourse.masks import make_identity

B, H, W, C = 32, 32, 32, 512
N = H * W                      # 1024 positions
G = 32                         # groups
GS = C // G                    # 16 channels per group
EPS = 1e-5
NCORES = 8
BPC = B // NCORES              # 4 examples per core
ISQ = float(1.0 / np.sqrt(C))  # score scale (folded into M' on host)

F32 = mybir.dt.float32
BF16 = mybir.dt.bfloat16
FP8 = mybir.dt.float8e4
AF = mybir.ActivationFunctionType
ALU = mybir.AluOpType
MS = bass.MemorySpace
DR = mybir.MatmulPerfMode.DoubleRow

SDT = BF16                     # score path (zt/gt/mq/wvp) dtype
ODT = FP8                      # attention-weight path (et/v) dtype
XT_DT = BF16                   # x^T / stats path stays bf16
SCALE_M = 512.0                # host upscale on M'; exp scale undoes it
SCALE_V = 1.0                  # v path scale (1.0: bf16 proj, fp8 storage)


class Ctx:
    pass


def _load_x(g, bi, nq=2):
    xn = g.xn_p.tile([128, 8, 512], F32, tag="xn", name=f"xn{bi}")
    qs = [g.nc.sync, g.nc.scalar, g.nc.gpsimd][:nq]
    for d in range(8):
        qs[d % nq].dma_start(xn[:, d, :], g.xr[bi, :, d, :])
    return xn


def _tr_group(g, bi, xn, xt, st6, t, half):
    """One transpose group: 4 PE transpose MMs -> PSUM -> xT copy -> stats."""
    nc = g.nc
    ps = g.pm.tile([128, 512], F32, tag="pm", name=f"ps_tr{bi}_{t}_{half}")
    for q in range(4):
        i = half * 4 + q
        nc.tensor.matmul(
            ps[:, q * 128:(q + 1) * 128],
            xn[:, i, t * 128:(t + 1) * 128],
            g.ident,
            is_transpose=True,
            start=(q == 0),
            stop=(q == 3),
        )
    nc.vector.tensor_copy(xt[:, t, half * 512:(half + 1) * 512], ps)
    nc.vector.bn_stats(st6[:, t, half, :], xt[:, t, half * 512:(half + 1) * 512])


def _stats_aggr(g, bi, st6):
    """DVE-only part: aggregate bn stats into m2 = [mean, E[x^2]]."""
    nc = g.nc
    mv = g.small.tile([128, 4, 2], F32, tag="mv", name=f"mv{bi}")
    for t in range(4):
        nc.vector.bn_aggr(mv[:, t, :], st6[:, t, :, :])
    m2 = g.small.tile([128, 4, 2], F32, tag="m2", name=f"m2{bi}")
    nc.vector.tensor_copy(m2[:, :, 0:1], mv[:, :, 0:1])
    nc.vector.tensor_mul(m2[:, :, 1:2], mv[:, :, 0:1], mv[:, :, 0:1])
    nc.vector.tensor_add(m2[:, :, 1:2], m2[:, :, 1:2], mv[:, :, 1:2])
    return m2


def _stats_pool(g, bi, m2):
    """Pool over the 16-channel groups (ONE tiny PE matmul) and produce
    per-group [rstd, mean] on 8 partitions."""
    nc = g.nc
    ps_g = g.aux.tile([8, 4, 2], F32, tag="aux", name=f"ps_g{bi}")
    nc.tensor.matmul(ps_g, g.a_pool, m2, start=True, stop=True)
    pg = g.small.tile([8, 4, 2], F32, tag="pg", name=f"pg{bi}")
    nc.vector.tensor_copy(pg, ps_g)
    vr = g.small.tile([8, 4, 1], F32, tag="vr", name=f"vr{bi}")
    nc.vector.tensor_mul(vr, pg[:, :, 0:1], pg[:, :, 0:1])
    nc.vector.tensor_sub(vr, pg[:, :, 1:2], vr)
    nc.scalar.activation(vr, vr, AF.Sqrt, bias=g.eps_c[:8])
    nc.vector.reciprocal(vr, vr)
    gab = g.small.tile([8, 4, 2], F32, tag="gab", name=f"gab{bi}")
    nc.vector.tensor_copy(gab[:, :, 0:1], vr)
    nc.vector.tensor_copy(gab[:, :, 1:2], pg[:, :, 0:1])
    return gab


def _stats_norm(g, bi, xt, gab):
    """Expand group stats to channels (ONE tiny PE matmul) + normalize."""
    nc = g.nc
    ps_ab = g.aux.tile([128, 4, 2], F32, tag="aux", name=f"ps_ab{bi}")
    nc.tensor.matmul(ps_ab, g.e8, gab, start=True, stop=True)
    # A = rstd * gn_scale ; Bb = gn_bias - mean * A
    ab = g.small.tile([128, 4, 2], F32, tag="ab", name=f"ab{bi}")
    tmpc = g.small.tile([128, 4, 1], F32, tag="tmpc", name=f"tmpc{bi}")
    nc.vector.tensor_mul(ab[:, :, 0:1], ps_ab[:, :, 0:1], g.gns_sb[:, :, 0:1])
    nc.vector.tensor_mul(tmpc, ps_ab[:, :, 1:2], ab[:, :, 0:1])
    nc.vector.tensor_sub(ab[:, :, 1:2], g.gnb_sb[:, :, 0:1], tmpc)
    zt = g.zt_p.tile([128, 4, 1024], SDT, tag="zt", name=f"zt{bi}")
    for t in range(4):
        nc.vector.tensor_scalar(
            out=zt[:, t, :], in0=xt[:, t, :],
            scalar1=ab[:, t, 0:1], scalar2=ab[:, t, 1:2],
            op0=ALU.mult, op1=ALU.add,
        )
    return zt


def _gv_stage(g, bi, zt, tr=None):
    """G^T = M'^T Z^T and V' = Z Wv' (natural); interleave the next
    example's transpose groups (dependency-free PE work)."""
    nc = g.nc
    gt = g.gt_p.tile([128, 4, 1024], SDT, tag="gt")
    for m in range(4):
        ps = [g.pm.tile([128, 512], F32, tag="pm", name=f"ps_g{bi}_{m}_{h}")
              for h in range(2)]
        for kk in range(4):
            for h in range(2):
                nc.tensor.matmul(
                    ps[h],
                    g.mq_sb[:, kk, m * 128:(m + 1) * 128],
                    zt[:, kk, h * 512:(h + 1) * 512],
                    start=(kk == 0),
                    stop=(kk == 3),
                )
        for h in range(2):
            nc.scalar.copy(gt[:, m, h * 512:(h + 1) * 512], ps[h])
    v = g.v_p.tile([128, 8, 512], ODT, tag="v")
    for i in range(8):
        ps = g.pm.tile([128, 512], F32, tag="pm")
        for kk in range(4):
            nc.tensor.matmul(
                ps,
                zt[:, kk, i * 128:(i + 1) * 128],
                g.wvp_sb[:, kk, :],
                start=(kk == 0),
                stop=(kk == 3),
            )
        nc.scalar.copy(v[:, i, :], ps)
    return gt, v


def _u_stage(g, bi, zt):
    """Per-key bias u_j = uvec . z_j  (only when b_qkv != 0)."""
    nc = g.nc
    ps_u = g.aux.tile([128, 8], F32, tag="aux", name=f"ps_u{bi}")
    for j in range(8):
        for kk in range(4):
            nc.tensor.matmul(
                ps_u[:, j:j + 1],
                zt[:, kk, j * 128:(j + 1) * 128],
                g.uv_sb[:, kk:kk + 1],
                start=(kk == 0),
                stop=(kk == 3),
            )
    u_sb = g.small.tile([128, 8], F32, tag="u_sb", name=f"u_sb{bi}")
    nc.vector.tensor_scalar(out=u_sb, in0=ps_u, scalar1=1.0, scalar2=-2.0,
                            op0=ALU.mult, op1=ALU.add)
    return u_sb


def _phase_st(g, bi, zt, gt, u_sb=None, tr=None):
    """Transposed scores + exp -> ET; optionally interleave the next
    example's transpose groups (dependency-free PE work).  Softmax
    denominators accumulate via M=1 ones-lhsT DoubleRow matmuls (trivial
    weight load) into a [1, 1024] row as the ET chunks appear."""
    nc = g.nc
    et = g.et_p.tile([128, 8, 1024], ODT, tag="et")
    s_ps = [g.aux.tile([1, 512], F32, tag="aux", name=f"s_ps{bi}_{h}")
            for h in range(2)]
    for j in range(8):
        ps = [g.pm.tile([128, 512], F32, tag="pm", name=f"ps_s{bi}_{j}_{h}")
              for h in range(2)]
        for ct in range(4):
            for h in range(2):
                nc.tensor.matmul(
                    ps[h],
                    zt[:, ct, j * 128:(j + 1) * 128],
                    gt[:, ct, h * 512:(h + 1) * 512],
                    start=(ct == 0),
                    stop=(ct == 3),
                )
        for h in range(2):
            nc.scalar.activation(
                et[:, j, h * 512:(h + 1) * 512], ps[h], AF.Exp,
                scale=1.0 / SCALE_M,
                bias=g.neg2 if u_sb is None else u_sb[:, j:j + 1])
        if tr is not None and j < 4:
            tr(2 * j)
            tr(2 * j + 1)
        if j % 2 == 1:
            jj = j // 2
            for h in range(2):
                nc.tensor.matmul(
                    s_ps[h],
                    g.ones2[:, :, 0:1],
                    et[:, 2 * jj:2 * jj + 2, h * 512:(h + 1) * 512],
                    start=(jj == 0),
                    stop=(jj == 3),
                    perf_mode=DR,
                )
    # denominator row -> column layout via a DRAM bounce; recip scale
    s_sb = g.small.tile([1, 1024], F32, tag="s_sb", name=f"s_sb{bi}")
    for h in range(2):
        nc.vector.tensor_copy(s_sb[:, h * 512:(h + 1) * 512], s_ps[h])
    s_dram = g.dram.tile([1, 1024], F32, tag="s_dram", name=f"s_dram{bi}")
    nc.sync.dma_start(s_dram, s_sb)
    s_col = g.small.tile([128, 8], F32, tag="s_col", name=f"s_col{bi}")
    nc.sync.dma_start(s_col, s_dram.rearrange("o (t p) -> p (o t)", p=128))
    rc = g.small.tile([128, 8], F32, tag="rc", name=f"rc{bi}")
    nc.vector.reciprocal(rc, s_col)
    return et, rc


def _phase_o(g, bi, xn, v, et, rc, mid1=None, mid2=None):
    """O natural, residual, store.  mid1/mid2 emit the next example's
    stats matmuls at points where their DVE inputs are already complete
    (no PE-FIFO stall)."""
    nc = g.nc
    res = g.res_p.tile([128, 8, 512], F32, tag="res")
    out_q = [nc.sync, nc.scalar]
    for i in range(8):
        ps_o = g.pm.tile([128, 512], F32, tag="pm", name=f"ps_o{bi}_{i}")
        for jj in range(4):
            nc.tensor.matmul(
                ps_o,
                et[:, 2 * jj:2 * jj + 2, i * 128:(i + 1) * 128],
                v[:, 2 * jj:2 * jj + 2, :],
                start=(jj == 0),
                stop=(jj == 3),
                perf_mode=DR,
            )
        # b' pre-add into xn (after the transposes/stats read xn)
        nc.gpsimd.tensor_add(xn[:, i, :], xn[:, i, :], g.bres_bc)
        nc.vector.scalar_tensor_tensor(
            out=res[:, i, :], in0=ps_o, scalar=rc[:, i:i + 1],
            in1=xn[:, i, :], op0=ALU.mult, op1=ALU.add,
        )
        out_q[i % 2].dma_start(g.outr[bi, :, i, :], res[:, i, :])
        if i == 0 and mid1 is not None:
            mid1()
        if i == 1 and mid2 is not None:
            mid2()


def build_program(has_u):
    nc = bacc.Bacc("TRN2", target_bir_lowering=False, debug=False)

    x_d = nc.dram_tensor("x", [BPC, N, C], F32, kind="ExternalInput")
    mq_d = nc.dram_tensor("m_qk", [C, C], SDT, kind="ExternalInput")
    wvp_d = nc.dram_tensor("w_vp", [C, C], SDT, kind="ExternalInput")
    bres_d = nc.dram_tensor("b_res", [C], F32, kind="ExternalInput")
    gns_d = nc.dram_tensor("gn_scale", [C], F32, kind="ExternalInput")
    gnb_d = nc.dram_tensor("gn_bias", [C], F32, kind="ExternalInput")
    if has_u:
        uv_d = nc.dram_tensor("u_vec", [C], F32, kind="ExternalInput")
    out_d = nc.dram_tensor("out", [BPC, N, C], F32, kind="ExternalOutput")

    g = Ctx()
    g.nc = nc
    g.xr = x_d.ap().rearrange("b (i p) c -> b p i c", p=128)
    g.outr = out_d.ap().rearrange("b (i p) c -> b p i c", p=128)

    with tile.TileContext(nc) as tc:
        from contextlib import ExitStack
        with ExitStack() as ctx:
            const = ctx.enter_context(tc.tile_pool(name="const", bufs=1))
            g.pm = ctx.enter_context(tc.tile_pool(name="pm", bufs=6, space=MS.PSUM))
            g.aux = ctx.enter_context(tc.tile_pool(name="aux", bufs=2, space=MS.PSUM))
            g.xn_p = ctx.enter_context(tc.tile_pool(name="xn", bufs=2))
            g.xt_p = ctx.enter_context(tc.tile_pool(name="xtp", bufs=1))
            g.zt_p = ctx.enter_context(tc.tile_pool(name="ztp", bufs=2))
            g.gt_p = ctx.enter_context(tc.tile_pool(name="gtp", bufs=1))
            g.v_p = ctx.enter_context(tc.tile_pool(name="vp", bufs=2))
            g.et_p = ctx.enter_context(tc.tile_pool(name="etp", bufs=2))
            g.res_p = ctx.enter_context(tc.tile_pool(name="resp", bufs=1))
            g.small = ctx.enter_context(tc.tile_pool(name="small", bufs=3))
            g.dram = ctx.enter_context(tc.tile_pool(name="dram", bufs=2,
                                                    space=MS.DRAM))

            # ---- example-0 input DMA first: it is on the critical path
            xn0 = _load_x(g, 0, nq=3)

            # ---- constants ----------------------------------------------
            g.ident = const.tile([128, 128], F32)
            make_identity(nc, g.ident)

            # PE warmup: real matmuls with no DMA dependency, issued while
            # the input DMAs run, so the HAM clock gate reaches K=8/8
            # before the first productive matmul.
            def warm(n, salt=[0]):
                for _ in range(n):
                    salt[0] += 1
                    ps_w = g.pm.tile([128, 512], F32, tag="pm",
                                     name=f"ps_w{salt[0]}")
                    nc.tensor.matmul(ps_w[:, 0:128], g.ident, g.ident,
                                     start=True, stop=True)
            g.warm = warm
            warm(24)

            g.a_pool = const.tile([128, 8], F32)
            nc.gpsimd.memset(g.a_pool, 1.0 / GS)
            nc.gpsimd.affine_select(
                out=g.a_pool, in_=g.a_pool, compare_op=ALU.is_ge, fill=0.0,
                base=0, pattern=[[-GS, 8]], channel_multiplier=1)
            nc.gpsimd.affine_select(
                out=g.a_pool, in_=g.a_pool, compare_op=ALU.is_ge, fill=0.0,
                base=GS - 1, pattern=[[GS, 8]], channel_multiplier=-1)

            g.e8 = const.tile([8, 128], F32)
            nc.gpsimd.memset(g.e8, 1.0)
            nc.gpsimd.affine_select(
                out=g.e8, in_=g.e8, compare_op=ALU.is_ge, fill=0.0,
                base=0, pattern=[[1, 128]], channel_multiplier=-GS)
            nc.gpsimd.affine_select(
                out=g.e8, in_=g.e8, compare_op=ALU.is_ge, fill=0.0,
                base=GS - 1, pattern=[[-1, 128]], channel_multiplier=GS)

            g.ones2 = const.tile([128, 2, 16], ODT)
            nc.vector.memset(g.ones2, SCALE_V)
            g.neg2 = const.tile([128, 1], F32)
            nc.vector.memset(g.neg2, -2.0)
            g.eps_c = const.tile([128, 1], F32)
            nc.vector.memset(g.eps_c, EPS)

            g.mq_sb = const.tile([128, 4, C], SDT)
            mqr = mq_d.ap().rearrange("(t p) d -> t p d", p=128)
            for t in range(4):
                nc.gpsimd.dma_start(g.mq_sb[:, t, :], mqr[t])
            g.wvp_sb = const.tile([128, 4, C], SDT)
            wvr = wvp_d.ap().rearrange("(t p) d -> t p d", p=128)
            for t in range(4):
                nc.gpsimd.dma_start(g.wvp_sb[:, t, :], wvr[t])

            g.gns_sb = const.tile([128, 4, 1], F32)
            nc.sync.dma_start(g.gns_sb[:, :, 0:1],
                              gns_d.ap().rearrange("(t p) -> p t", p=128))
            g.gnb_sb = const.tile([128, 4, 1], F32)
            nc.sync.dma_start(g.gnb_sb[:, :, 0:1],
                              gnb_d.ap().rearrange("(t p) -> p t", p=128))

            def bcast(src_ap):
                return bass.AP(
                    tensor=src_ap.tensor, offset=src_ap.offset,
                    ap=[[0, 128]] + [list(p) for p in src_ap.ap])

            g.bres_bc = const.tile([128, 512], F32)
            nc.gpsimd.dma_start(g.bres_bc, bcast(bres_d.ap()))

            if has_u:
                uvf = const.tile([128, 4], F32)
                nc.sync.dma_start(uvf, uv_d.ap().rearrange("(t p) -> p t", p=128))
                g.uv_sb = const.tile([128, 4], SDT)
                nc.vector.tensor_copy(g.uv_sb, uvf)

            # ---- example-0 prologue -------------------------------------
            xt0 = g.xt_p.tile([128, 4, 1024], XT_DT, tag="xt", name="xt0")
            st6_0 = g.small.tile([128, 4, 2, 6], F32, tag="st6", name="st6_0")
            for t in range(4):
                for half in range(2):
                    _tr_group(g, 0, xn0, xt0, st6_0, t, half)
            warm(90)
            m2_0 = _stats_aggr(g, 0, st6_0)
            gab0 = _stats_pool(g, 0, m2_0)
            warm(16)
            zt0 = _stats_norm(g, 0, xt0, gab0)
            warm(16)

            # ---- pipelined per-example emission -------------------------
            state = (xn0, zt0)
            nxt = {}
            for bi in range(BPC):
                xn, zt = state
                tr = None
                if bi + 1 < BPC:
                    xn1 = _load_x(g, bi + 1)
                    xt1 = g.xt_p.tile([128, 4, 1024], XT_DT, tag="xt",
                                      name=f"xt{bi+1}")
                    st6 = g.small.tile([128, 4, 2, 6], F32, tag="st6",
                                       name=f"st6_{bi+1}")

                    def tr(j, xn1=xn1, xt1=xt1, st6=st6, b1=bi + 1):
                        _tr_group(g, b1, xn1, xt1, st6, j // 2, j % 2)
                gt, v = _gv_stage(g, bi, zt)
                u_sb = _u_stage(g, bi, zt) if has_u else None
                et, rc = _phase_st(g, bi, zt, gt, u_sb, tr)
                mid1 = mid2 = None
                if bi + 1 < BPC:
                    # aggr first in the DVE FIFO, ahead of the O-phase
                    # stt ops; pool/norm matmuls fire once inputs exist
                    nxt["m2"] = _stats_aggr(g, bi + 1, st6)

                    def mid1(b1=bi + 1):
                        nxt["gab"] = _stats_pool(g, b1, nxt["m2"])

                    def mid2(xt1=xt1, b1=bi + 1):
                        nxt["zt"] = _stats_norm(g, b1, xt1, nxt["gab"])
                _phase_o(g, bi, xn, v, et, rc, mid1, mid2)
                if bi + 1 < BPC:
                    state = (xn1, nxt["zt"])

    nc.compile()
    return nc


_NC = {}


def _get_nc(has_u):
    if has_u not in _NC:
        _NC[has_u] = build_program(has_u)
    return _NC[has_u]


def kernel(x, t, gn_scale, gn_bias, w_qkv, b_qkv, w_out, b_out):
    import ml_dtypes
    x = np.ascontiguousarray(np.asarray(x, np.float32).reshape(B, N, C))
    w_qkv = np.asarray(w_qkv, np.float32)
    b_qkv = np.asarray(b_qkv, np.float32)
    w_out = np.asarray(w_out, np.float32)
    b_out = np.asarray(b_out, np.float32)
    wq, wk, wv = w_qkv[:, 0:C], w_qkv[:, C:2 * C], w_qkv[:, 2 * C:3 * C]
    bq, bv = b_qkv[0:C], b_qkv[2 * C:3 * C]

    m_qk = ((wq @ wk.T) * (ISQ * SCALE_M)).astype(ml_dtypes.bfloat16)
    w_vp = ((wv @ w_out) * SCALE_V).astype(ml_dtypes.bfloat16)
    b_res = (bv @ w_out + b_out).astype(np.float32)
    u_vec = ((wk @ bq) * ISQ).astype(np.float32)
    has_u = bool(np.any(u_vec != 0.0))

    shared = {
        "m_qk": np.ascontiguousarray(m_qk),
        "w_vp": np.ascontiguousarray(w_vp),
        "b_res": np.ascontiguousarray(b_res),
        "gn_scale": np.ascontiguousarray(np.asarray(gn_scale, np.float32)),
        "gn_bias": np.ascontiguousarray(np.asarray(gn_bias, np.float32)),
    }
    if has_u:
        shared["u_vec"] = np.ascontiguousarray(u_vec)
    in_maps = [
        {"x": x[c * BPC:(c + 1) * BPC], **shared} for c in range(NCORES)
    ]
    nc = _get_nc(has_u)
    res = run_bass_kernel_spmd(nc, in_maps, core_ids=list(range(NCORES)))
    out = np.concatenate([res.results[c]["out"] for c in range(NCORES)], axis=0)
    return out.reshape(B, H, W, C)


# revision 31
# speedup vs baseline: 1.1380x; 1.1380x over previous
"""TRN2 Bass kernel for nn_Attention_20444044329649.

GroupNorm(32) -> qkv dense -> single-head spatial attention (1024 pos) ->
out dense -> residual.  B=32 examples sharded 4-per-core across 8 cores;
params replicated.

v3 — v2's algebraic folds plus PE-FIFO discipline:

  * scores:  S*isq = Z M' Z^T with M' = isq*Wq Wk^T host-precomputed;
    device computes G^T = M'^T Z^T only (no K projection).  q/k biases:
    per-query term cancels in softmax; per-key term u_j applied as a
    per-partition exp bias (only emitted when b_qkv != 0).
  * out-proj fold: Wv' = Wv W_out, b' = bv W_out + b_out; O computed in
    NATURAL layout via lhsT=ET chunks, rhs=V'.
  * softmax denominators: N=1 matmuls interleaved with the O matmuls.
  * big matmul operands bf16; accumulation fp32 in PSUM; residual fp32.
  * GroupNorm stats batched to exactly TWO tiny PE matmuls per example
    (pool + expand across all 4 channel chunks at once), emitted inside
    the PREVIOUS example's O phase where their DVE inputs are already
    complete — the PE strict-FIFO queue never waits on the serial DVE
    stats chain.
  * next example's x transposes interleaved into the ST j-loop: no-dep
    PE work that keeps the HAM activity window busy (transpose-mode gaps
    plus boundary stalls previously re-throttled the PE to 1.2 GHz for
    ~10 us every example).
  * zt/v/et double-buffered so cross-example WAR hazards never
    serialize; example-0 input DMA spread over 4 queues.
"""

import numpy as np

import concourse.bass as bass
import concourse.mybir as mybir
import concourse.tile as tile
from concourse import bacc
from concourse.bass_utils import run_bass_kernel_spmd
from concourse.masks import make_identity

B, H, W, C = 32, 32, 32, 512
N = H * W                      # 1024 positions
G = 32                         # groups
GS = C // G                    # 16 channels per group
EPS = 1e-5
NCORES = 8
BPC = B // NCORES              # 4 examples per core
ISQ = float(1.0 / np.sqrt(C))  # score scale (folded into M' on host)

F32 = mybir.dt.float32
BF16 = mybir.dt.bfloat16
FP8 = mybir.dt.float8e4
AF = mybir.ActivationFunctionType
ALU = mybir.AluOpType
MS = bass.MemorySpace
DR = mybir.MatmulPerfMode.DoubleRow

SDT = BF16                     # score path (zt/gt/mq/wvp) dtype
ODT = FP8                      # attention-weight path (et/v) dtype
XT_DT = BF16                   # x^T / stats path stays bf16
SCALE_M = 512.0                # host upscale on M'; exp scale undoes it
SCALE_V = 1.0                  # v path scale (1.0: bf16 proj, fp8 storage)


class Ctx:
    pass


def _load_x(g, bi, nq=2):
    xn = g.xn_p.tile([128, 8, 512], F32, tag="xn", name=f"xn{bi}")
    qs = [g.nc.sync, g.nc.scalar, g.nc.gpsimd][:nq]
    for d in range(8):
        qs[d % nq].dma_start(xn[:, d, :], g.xr[bi, :, d, :])
    return xn


def _tr_group(g, bi, xn, xt, st6, t, half):
    """One transpose group: 4 PE transpose MMs -> PSUM -> xT copy -> stats."""
    nc = g.nc
    ps = g.pm.tile([128, 512], F32, tag="pm", name=f"ps_tr{bi}_{t}_{half}")
    for q in range(4):
        i = half * 4 + q
        nc.tensor.matmul(
            ps[:, q * 128:(q + 1) * 128],
            xn[:, i, t * 128:(t + 1) * 128],
            g.ident,
            is_transpose=True,
            start=(q == 0),
            stop=(q == 3),
        )
    nc.vector.tensor_copy(xt[:, t, half * 512:(half + 1) * 512], ps)
    nc.vector.bn_stats(st6[:, t, half, :], xt[:, t, half * 512:(half + 1) * 512])


def _stats_aggr(g, bi, st6):
    """DVE-only part: aggregate bn stats into m2 = [mean, E[x^2]]."""
    nc = g.nc
    mv = g.small.tile([128, 4, 2], F32, tag="mv", name=f"mv{bi}")
    for t in range(4):
        nc.vector.bn_aggr(mv[:, t, :], st6[:, t, :, :])
    m2 = g.small.tile([128, 4, 2], F32, tag="m2", name=f"m2{bi}")
    nc.vector.tensor_copy(m2[:, :, 0:1], mv[:, :, 0:1])
    nc.vector.tensor_mul(m2[:, :, 1:2], mv[:, :, 0:1], mv[:, :, 0:1])
    nc.vector.tensor_add(m2[:, :, 1:2], m2[:, :, 1:2], mv[:, :, 1:2])
    return m2


def _stats_pool(g, bi, m2):
    """Pool over the 16-channel groups (ONE tiny PE matmul) and produce
    per-group [rstd, mean] on 8 partitions."""
    nc = g.nc
    ps_g = g.aux.tile([8, 4, 2], F32, tag="aux", name=f"ps_g{bi}")
    nc.tensor.matmul(ps_g, g.a_pool, m2, start=True, stop=True)
    pg = g.small.tile([8, 4, 2], F32, tag="pg", name=f"pg{bi}")
    nc.vector.tensor_copy(pg, ps_g)
    vr = g.small.tile([8, 4, 1], F32, tag="vr", name=f"vr{bi}")
    nc.vector.tensor_mul(vr, pg[:, :, 0:1], pg[:, :, 0:1])
    nc.vector.tensor_sub(vr, pg[:, :, 1:2], vr)
    nc.scalar.activation(vr, vr, AF.Sqrt, bias=g.eps_c[:8])
    nc.vector.reciprocal(vr, vr)
    gab = g.small.tile([8, 4, 2], F32, tag="gab", name=f"gab{bi}")
    nc.vector.tensor_copy(gab[:, :, 0:1], vr)
    nc.vector.tensor_copy(gab[:, :, 1:2], pg[:, :, 0:1])
    return gab


def _stats_norm(g, bi, xt, gab):
    """Expand group stats to channels (ONE tiny PE matmul) + normalize."""
    nc = g.nc
    ps_ab = g.aux.tile([128, 4, 2], F32, tag="aux", name=f"ps_ab{bi}")
    nc.tensor.matmul(ps_ab, g.e8, gab, start=True, stop=True)
    # A = rstd * gn_scale ; Bb = gn_bias - mean * A
    ab = g.small.tile([128, 4, 2], F32, tag="ab", name=f"ab{bi}")
    tmpc = g.small.tile([128, 4, 1], F32, tag="tmpc", name=f"tmpc{bi}")
    nc.vector.tensor_mul(ab[:, :, 0:1], ps_ab[:, :, 0:1], g.gns_sb[:, :, 0:1])
    nc.vector.tensor_mul(tmpc, ps_ab[:, :, 1:2], ab[:, :, 0:1])
    nc.vector.tensor_sub(ab[:, :, 1:2], g.gnb_sb[:, :, 0:1], tmpc)
    zt = g.zt_p.tile([128, 4, 1024], SDT, tag="zt", name=f"zt{bi}")
    for t in range(4):
        if t % 2 == 0:
            nc.vector.tensor_scalar(
                out=zt[:, t, :], in0=xt[:, t, :],
                scalar1=ab[:, t, 0:1], scalar2=ab[:, t, 1:2],
                op0=ALU.mult, op1=ALU.add,
            )
        else:
            nc.scalar.activation(
                zt[:, t, :], xt[:, t, :], AF.Identity,
                scale=ab[:, t, 0:1], bias=ab[:, t, 1:2])
    return zt


def _gv_stage(g, bi, zt, tr=None):
    """G^T = M'^T Z^T and V' = Z Wv' (natural); interleave the next
    example's transpose groups (dependency-free PE work)."""
    nc = g.nc
    gt = g.gt_p.tile([128, 4, 1024], SDT, tag="gt")
    for m in range(4):
        ps = [g.pm.tile([128, 512], F32, tag="pm", name=f"ps_g{bi}_{m}_{h}")
              for h in range(2)]
        for kk in range(4):
            for h in range(2):
                nc.tensor.matmul(
                    ps[h],
                    g.mq_sb[:, kk, m * 128:(m + 1) * 128],
                    zt[:, kk, h * 512:(h + 1) * 512],
                    start=(kk == 0),
                    stop=(kk == 3),
                )
        for h in range(2):
            nc.scalar.copy(gt[:, m, h * 512:(h + 1) * 512], ps[h])
    v = g.v_p.tile([128, 8, 512], ODT, tag="v")
    for i in range(8):
        ps = g.pm.tile([128, 512], F32, tag="pm")
        for kk in range(4):
            nc.tensor.matmul(
                ps,
                zt[:, kk, i * 128:(i + 1) * 128],
                g.wvp_sb[:, kk, :],
                start=(kk == 0),
                stop=(kk == 3),
            )
        nc.scalar.copy(v[:, i, :], ps)
    return gt, v


def _u_stage(g, bi, zt):
    """Per-key bias u_j = uvec . z_j  (only when b_qkv != 0)."""
    nc = g.nc
    ps_u = g.aux.tile([128, 8], F32, tag="aux", name=f"ps_u{bi}")
    for j in range(8):
        for kk in range(4):
            nc.tensor.matmul(
                ps_u[:, j:j + 1],
                zt[:, kk, j * 128:(j + 1) * 128],
                g.uv_sb[:, kk:kk + 1],
                start=(kk == 0),
                stop=(kk == 3),
            )
    u_sb = g.small.tile([128, 8], F32, tag="u_sb", name=f"u_sb{bi}")
    nc.vector.tensor_scalar(out=u_sb, in0=ps_u, scalar1=1.0, scalar2=-2.0,
                            op0=ALU.mult, op1=ALU.add)
    return u_sb


def _phase_st(g, bi, zt, gt, u_sb=None, tr=None):
    """Transposed scores + exp -> ET; optionally interleave the next
    example's transpose groups (dependency-free PE work).  Softmax
    denominators accumulate via M=1 ones-lhsT DoubleRow matmuls (trivial
    weight load) into a [1, 1024] row as the ET chunks appear."""
    nc = g.nc
    et = g.et_p.tile([128, 8, 1024], ODT, tag="et")
    s_ps = [g.aux.tile([1, 512], F32, tag="aux", name=f"s_ps{bi}_{h}")
            for h in range(2)]
    for j in range(8):
        ps = [g.pm.tile([128, 512], F32, tag="pm", name=f"ps_s{bi}_{j}_{h}")
              for h in range(2)]
        for ct in range(4):
            for h in range(2):
                nc.tensor.matmul(
                    ps[h],
                    zt[:, ct, j * 128:(j + 1) * 128],
                    gt[:, ct, h * 512:(h + 1) * 512],
                    start=(ct == 0),
                    stop=(ct == 3),
                )
        for h in range(2):
            nc.scalar.activation(
                et[:, j, h * 512:(h + 1) * 512], ps[h], AF.Exp,
                scale=1.0 / SCALE_M,
                bias=g.neg2 if u_sb is None else u_sb[:, j:j + 1])
        if tr is not None:
            tr(j)
        if j % 2 == 1:
            jj = j // 2
            for h in range(2):
                nc.tensor.matmul(
                    s_ps[h],
                    g.ones2[:, :, 0:1],
                    et[:, 2 * jj:2 * jj + 2, h * 512:(h + 1) * 512],
                    start=(jj == 0),
                    stop=(jj == 3),
                    perf_mode=DR,
                )
    return et, s_ps


def _denom(g, bi, s_ps):
    """Denominator row -> column layout via a DRAM bounce; recip scale."""
    nc = g.nc
    s_sb = g.small.tile([1, 1024], F32, tag="s_sb", name=f"s_sb{bi}")
    for h in range(2):
        nc.vector.tensor_copy(s_sb[:, h * 512:(h + 1) * 512], s_ps[h])
    s_dram = g.dram.tile([1, 1024], F32, tag="s_dram", name=f"s_dram{bi}")
    nc.sync.dma_start(s_dram, s_sb)
    s_col = g.small.tile([128, 8], F32, tag="s_col", name=f"s_col{bi}")
    nc.sync.dma_start(s_col, s_dram.rearrange("o (t p) -> p (o t)", p=128))
    rc = g.small.tile([128, 8], F32, tag="rc", name=f"rc{bi}")
    nc.vector.reciprocal(rc, s_col)
    return rc


def _phase_o(g, bi, xn, v, et, rc, mid1=None, mid2=None):
    """O natural, residual, store.  mid1/mid2 emit the next example's
    stats matmuls at points where their DVE inputs are already complete
    (no PE-FIFO stall)."""
    nc = g.nc
    res = g.res_p.tile([128, 8, 512], F32, tag="res")
    out_q = [nc.sync, nc.scalar]
    for i in range(8):
        ps_o = g.pm.tile([128, 512], F32, tag="pm", name=f"ps_o{bi}_{i}")
        for jj in range(4):
            nc.tensor.matmul(
                ps_o,
                et[:, 2 * jj:2 * jj + 2, i * 128:(i + 1) * 128],
                v[:, 2 * jj:2 * jj + 2, :],
                start=(jj == 0),
                stop=(jj == 3),
                perf_mode=DR,
            )
        # b' pre-add into xn (after the transposes/stats read xn)
        nc.gpsimd.tensor_add(xn[:, i, :], xn[:, i, :], g.bres_bc)
        nc.vector.scalar_tensor_tensor(
            out=res[:, i, :], in0=ps_o, scalar=rc[:, i:i + 1],
            in1=xn[:, i, :], op0=ALU.mult, op1=ALU.add,
        )
        out_q[i % 2].dma_start(g.outr[bi, :, i, :], res[:, i, :])
        if i == 0 and mid1 is not None:
            mid1()
        if i == 1 and mid2 is not None:
            mid2()


def build_program(has_u):
    nc = bacc.Bacc("TRN2", target_bir_lowering=False, debug=False)

    x_d = nc.dram_tensor("x", [BPC, N, C], F32, kind="ExternalInput")
    mq_d = nc.dram_tensor("m_qk", [C, C], SDT, kind="ExternalInput")
    wvp_d = nc.dram_tensor("w_vp", [C, C], SDT, kind="ExternalInput")
    bres_d = nc.dram_tensor("b_res", [C], F32, kind="ExternalInput")
    gns_d = nc.dram_tensor("gn_scale", [C], F32, kind="ExternalInput")
    gnb_d = nc.dram_tensor("gn_bias", [C], F32, kind="ExternalInput")
    if has_u:
        uv_d = nc.dram_tensor("u_vec", [C], F32, kind="ExternalInput")
    out_d = nc.dram_tensor("out", [BPC, N, C], F32, kind="ExternalOutput")

    g = Ctx()
    g.nc = nc
    g.xr = x_d.ap().rearrange("b (i p) c -> b p i c", p=128)
    g.outr = out_d.ap().rearrange("b (i p) c -> b p i c", p=128)

    with tile.TileContext(nc) as tc:
        from contextlib import ExitStack
        with ExitStack() as ctx:
            const = ctx.enter_context(tc.tile_pool(name="const", bufs=1))
            g.pm = ctx.enter_context(tc.tile_pool(name="pm", bufs=6, space=MS.PSUM))
            g.aux = ctx.enter_context(tc.tile_pool(name="aux", bufs=2, space=MS.PSUM))
            g.xn_p = ctx.enter_context(tc.tile_pool(name="xn", bufs=2))
            g.xt_p = ctx.enter_context(tc.tile_pool(name="xtp", bufs=1))
            g.zt_p = ctx.enter_context(tc.tile_pool(name="ztp", bufs=2))
            g.gt_p = ctx.enter_context(tc.tile_pool(name="gtp", bufs=1))
            g.v_p = ctx.enter_context(tc.tile_pool(name="vp", bufs=2))
            g.et_p = ctx.enter_context(tc.tile_pool(name="etp", bufs=2))
            g.res_p = ctx.enter_context(tc.tile_pool(name="resp", bufs=1))
            g.small = ctx.enter_context(tc.tile_pool(name="small", bufs=3))
            g.dram = ctx.enter_context(tc.tile_pool(name="dram", bufs=2,
                                                    space=MS.DRAM))

            # ---- example-0 input DMA first: it is on the critical path
            xn0 = _load_x(g, 0, nq=3)

            # ---- constants ----------------------------------------------
            g.ident = const.tile([128, 128], F32)
            make_identity(nc, g.ident)

            # PE warmup: real matmuls with no DMA dependency, issued while
            # the input DMAs run, so the HAM clock gate reaches K=8/8
            # before the first productive matmul.
            def warm(n, salt=[0]):
                for _ in range(n):
                    salt[0] += 1
                    ps_w = g.pm.tile([128, 512], F32, tag="pm",
                                     name=f"ps_w{salt[0]}")
                    nc.tensor.matmul(ps_w[:, 0:128], g.ident, g.ident,
                                     start=True, stop=True)
            g.warm = warm
            warm(24)

            g.a_pool = const.tile([128, 8], F32)
            nc.gpsimd.memset(g.a_pool, 1.0 / GS)
            nc.gpsimd.affine_select(
                out=g.a_pool, in_=g.a_pool, compare_op=ALU.is_ge, fill=0.0,
                base=0, pattern=[[-GS, 8]], channel_multiplier=1)
            nc.gpsimd.affine_select(
                out=g.a_pool, in_=g.a_pool, compare_op=ALU.is_ge, fill=0.0,
                base=GS - 1, pattern=[[GS, 8]], channel_multiplier=-1)

            g.e8 = const.tile([8, 128], F32)
            nc.gpsimd.memset(g.e8, 1.0)
            nc.gpsimd.affine_select(
                out=g.e8, in_=g.e8, compare_op=ALU.is_ge, fill=0.0,
                base=0, pattern=[[1, 128]], channel_multiplier=-GS)
            nc.gpsimd.affine_select(
                out=g.e8, in_=g.e8, compare_op=ALU.is_ge, fill=0.0,
                base=GS - 1, pattern=[[-1, 128]], channel_multiplier=GS)

            g.ones2 = const.tile([128, 2, 16], ODT)
            nc.vector.memset(g.ones2, SCALE_V)
            g.neg2 = const.tile([128, 1], F32)
            nc.vector.memset(g.neg2, -2.0)
            g.eps_c = const.tile([128, 1], F32)
            nc.vector.memset(g.eps_c, EPS)

            g.mq_sb = const.tile([128, 4, C], SDT)
            mqr = mq_d.ap().rearrange("(t p) d -> t p d", p=128)
            for t in range(4):
                nc.gpsimd.dma_start(g.mq_sb[:, t, :], mqr[t])
            g.wvp_sb = const.tile([128, 4, C], SDT)
            wvr = wvp_d.ap().rearrange("(t p) d -> t p d", p=128)
            for t in range(4):
                nc.gpsimd.dma_start(g.wvp_sb[:, t, :], wvr[t])

            g.gns_sb = const.tile([128, 4, 1], F32)
            nc.sync.dma_start(g.gns_sb[:, :, 0:1],
                              gns_d.ap().rearrange("(t p) -> p t", p=128))
            g.gnb_sb = const.tile([128, 4, 1], F32)
            nc.sync.dma_start(g.gnb_sb[:, :, 0:1],
                              gnb_d.ap().rearrange("(t p) -> p t", p=128))

            def bcast(src_ap):
                return bass.AP(
                    tensor=src_ap.tensor, offset=src_ap.offset,
                    ap=[[0, 128]] + [list(p) for p in src_ap.ap])

            g.bres_bc = const.tile([128, 512], F32)
            nc.gpsimd.dma_start(g.bres_bc, bcast(bres_d.ap()))

            if has_u:
                uvf = const.tile([128, 4], F32)
                nc.sync.dma_start(uvf, uv_d.ap().rearrange("(t p) -> p t", p=128))
                g.uv_sb = const.tile([128, 4], SDT)
                nc.vector.tensor_copy(g.uv_sb, uvf)

            # ---- example-0 prologue -------------------------------------
            xt0 = g.xt_p.tile([128, 4, 1024], XT_DT, tag="xt", name="xt0")
            st6_0 = g.small.tile([128, 4, 2, 6], F32, tag="st6", name="st6_0")
            for t in range(4):
                for half in range(2):
                    _tr_group(g, 0, xn0, xt0, st6_0, t, half)
            warm(90)
            m2_0 = _stats_aggr(g, 0, st6_0)
            gab0 = _stats_pool(g, 0, m2_0)
            warm(16)
            zt0 = _stats_norm(g, 0, xt0, gab0)
            warm(16)

            # ---- pipelined per-example emission -------------------------
            state = (xn0, zt0)
            nxt = {}
            for bi in range(BPC):
                xn, zt = state
                tr = None
                if bi + 1 < BPC:
                    xn1 = _load_x(g, bi + 1)
                    xt1 = g.xt_p.tile([128, 4, 1024], XT_DT, tag="xt",
                                      name=f"xt{bi+1}")
                    st6 = g.small.tile([128, 4, 2, 6], F32, tag="st6",
                                       name=f"st6_{bi+1}")

                    def tr(j, xn1=xn1, xt1=xt1, st6=st6, b1=bi + 1):
                        _tr_group(g, b1, xn1, xt1, st6, j // 2, j % 2)
                gt, v = _gv_stage(g, bi, zt)
                u_sb = _u_stage(g, bi, zt) if has_u else None
                et, s_ps = _phase_st(g, bi, zt, gt, u_sb, tr)
                mid1 = mid2 = None
                if bi + 1 < BPC:
                    # aggr first in the DVE FIFO, ahead of the denominator
                    # bounce and the O-phase stt ops
                    nxt["m2"] = _stats_aggr(g, bi + 1, st6)
                rc = _denom(g, bi, s_ps)
                if bi + 1 < BPC:

                    def mid1(b1=bi + 1):
                        nxt["gab"] = _stats_pool(g, b1, nxt["m2"])

                    def mid2(xt1=xt1, b1=bi + 1):
                        nxt["zt"] = _stats_norm(g, b1, xt1, nxt["gab"])
                _phase_o(g, bi, xn, v, et, rc, mid1, mid2)
                if bi + 1 < BPC:
                    state = (xn1, nxt["zt"])

    nc.compile()
    return nc


_NC = {}


def _get_nc(has_u):
    if has_u not in _NC:
        _NC[has_u] = build_program(has_u)
    return _NC[has_u]


def kernel(x, t, gn_scale, gn_bias, w_qkv, b_qkv, w_out, b_out):
    import ml_dtypes
    x = np.ascontiguousarray(np.asarray(x, np.float32).reshape(B, N, C))
    w_qkv = np.asarray(w_qkv, np.float32)
    b_qkv = np.asarray(b_qkv, np.float32)
    w_out = np.asarray(w_out, np.float32)
    b_out = np.asarray(b_out, np.float32)
    wq, wk, wv = w_qkv[:, 0:C], w_qkv[:, C:2 * C], w_qkv[:, 2 * C:3 * C]
    bq, bv = b_qkv[0:C], b_qkv[2 * C:3 * C]

    m_qk = ((wq @ wk.T) * (ISQ * SCALE_M)).astype(ml_dtypes.bfloat16)
    w_vp = ((wv @ w_out) * SCALE_V).astype(ml_dtypes.bfloat16)
    b_res = (bv @ w_out + b_out).astype(np.float32)
    u_vec = ((wk @ bq) * ISQ).astype(np.float32)
    has_u = bool(np.any(u_vec != 0.0))

    shared = {
        "m_qk": np.ascontiguousarray(m_qk),
        "w_vp": np.ascontiguousarray(w_vp),
        "b_res": np.ascontiguousarray(b_res),
        "gn_scale": np.ascontiguousarray(np.asarray(gn_scale, np.float32)),
        "gn_bias": np.ascontiguousarray(np.asarray(gn_bias, np.float32)),
    }
    if has_u:
        shared["u_vec"] = np.ascontiguousarray(u_vec)
    in_maps = [
        {"x": x[c * BPC:(c + 1) * BPC], **shared} for c in range(NCORES)
    ]
    nc = _get_nc(has_u)
    res = run_bass_kernel_spmd(nc, in_maps, core_ids=list(range(NCORES)))
    out = np.concatenate([res.results[c]["out"] for c in range(NCORES)], axis=0)
    return out.reshape(B, H, W, C)


# revision 32
# speedup vs baseline: 1.1455x; 1.0066x over previous
"""TRN2 Bass kernel for nn_Attention_20444044329649.

GroupNorm(32) -> qkv dense -> single-head spatial attention (1024 pos) ->
out dense -> residual.  B=32 examples sharded 4-per-core across 8 cores;
params replicated.

v3 — v2's algebraic folds plus PE-FIFO discipline:

  * scores:  S*isq = Z M' Z^T with M' = isq*Wq Wk^T host-precomputed;
    device computes G^T = M'^T Z^T only (no K projection).  q/k biases:
    per-query term cancels in softmax; per-key term u_j applied as a
    per-partition exp bias (only emitted when b_qkv != 0).
  * out-proj fold: Wv' = Wv W_out, b' = bv W_out + b_out; O computed in
    NATURAL layout via lhsT=ET chunks, rhs=V'.
  * softmax denominators: N=1 matmuls interleaved with the O matmuls.
  * big matmul operands bf16; accumulation fp32 in PSUM; residual fp32.
  * GroupNorm stats batched to exactly TWO tiny PE matmuls per example
    (pool + expand across all 4 channel chunks at once), emitted inside
    the PREVIOUS example's O phase where their DVE inputs are already
    complete — the PE strict-FIFO queue never waits on the serial DVE
    stats chain.
  * next example's x transposes interleaved into the ST j-loop: no-dep
    PE work that keeps the HAM activity window busy (transpose-mode gaps
    plus boundary stalls previously re-throttled the PE to 1.2 GHz for
    ~10 us every example).
  * zt/v/et double-buffered so cross-example WAR hazards never
    serialize; example-0 input DMA spread over 4 queues.
"""

import numpy as np

import concourse.bass as bass
import concourse.mybir as mybir
import concourse.tile as tile
from concourse import bacc
from concourse.bass_utils import run_bass_kernel_spmd
from concourse.masks import make_identity

B, H, W, C = 32, 32, 32, 512
N = H * W                      # 1024 positions
G = 32                         # groups
GS = C // G                    # 16 channels per group
EPS = 1e-5
NCORES = 8
BPC = B // NCORES              # 4 examples per core
ISQ = float(1.0 / np.sqrt(C))  # score scale (folded into M' on host)

F32 = mybir.dt.float32
BF16 = mybir.dt.bfloat16
FP8 = mybir.dt.float8e4
AF = mybir.ActivationFunctionType
ALU = mybir.AluOpType
MS = bass.MemorySpace
DR = mybir.MatmulPerfMode.DoubleRow

SDT = BF16                     # score path (zt/gt/mq/wvp) dtype
ODT = FP8                      # attention-weight path (et/v) dtype
XT_DT = BF16                   # x^T / stats path stays bf16
SCALE_M = 512.0                # host upscale on M'; exp scale undoes it
SCALE_V = 1.0                  # v path scale (1.0: bf16 proj, fp8 storage)


class Ctx:
    pass


def _load_x(g, bi, nq=2):
    xn = g.xn_p.tile([128, 8, 512], F32, tag="xn", name=f"xn{bi}")
    qs = [g.nc.sync, g.nc.scalar, g.nc.gpsimd][:nq]
    for d in range(8):
        qs[d % nq].dma_start(xn[:, d, :], g.xr[bi, :, d, :])
    return xn


def _tr_group(g, bi, xn, xt, st6, t, half):
    """One transpose group: 4 PE transpose MMs -> PSUM -> xT copy -> stats."""
    nc = g.nc
    ps = g.pm.tile([128, 512], F32, tag="pm", name=f"ps_tr{bi}_{t}_{half}")
    for q in range(4):
        i = half * 4 + q
        nc.tensor.matmul(
            ps[:, q * 128:(q + 1) * 128],
            xn[:, i, t * 128:(t + 1) * 128],
            g.ident,
            is_transpose=True,
            start=(q == 0),
            stop=(q == 3),
        )
    nc.vector.tensor_copy(xt[:, t, half * 512:(half + 1) * 512], ps)
    nc.vector.bn_stats(st6[:, t, half, :], xt[:, t, half * 512:(half + 1) * 512])


def _stats_aggr(g, bi, st6):
    """DVE-only part: aggregate bn stats into m2 = [mean, E[x^2]]."""
    nc = g.nc
    mv = g.small.tile([128, 4, 2], F32, tag="mv", name=f"mv{bi}")
    for t in range(4):
        nc.vector.bn_aggr(mv[:, t, :], st6[:, t, :, :])
    m2 = g.small.tile([128, 4, 2], F32, tag="m2", name=f"m2{bi}")
    nc.vector.tensor_copy(m2[:, :, 0:1], mv[:, :, 0:1])
    nc.vector.tensor_mul(m2[:, :, 1:2], mv[:, :, 0:1], mv[:, :, 0:1])
    nc.vector.tensor_add(m2[:, :, 1:2], m2[:, :, 1:2], mv[:, :, 1:2])
    return m2


def _stats_pool(g, bi, m2):
    """Pool over the 16-channel groups (ONE tiny PE matmul) and produce
    per-group [rstd, mean] on 8 partitions."""
    nc = g.nc
    ps_g = g.aux.tile([8, 4, 2], F32, tag="aux", name=f"ps_g{bi}")
    nc.tensor.matmul(ps_g, g.a_pool, m2, start=True, stop=True)
    pg = g.small.tile([8, 4, 2], F32, tag="pg", name=f"pg{bi}")
    nc.vector.tensor_copy(pg, ps_g)
    vr = g.small.tile([8, 4, 1], F32, tag="vr", name=f"vr{bi}")
    nc.vector.tensor_mul(vr, pg[:, :, 0:1], pg[:, :, 0:1])
    nc.vector.tensor_sub(vr, pg[:, :, 1:2], vr)
    nc.scalar.activation(vr, vr, AF.Sqrt, bias=g.eps_c[:8])
    nc.vector.reciprocal(vr, vr)
    gab = g.small.tile([8, 4, 2], F32, tag="gab", name=f"gab{bi}")
    nc.vector.tensor_copy(gab[:, :, 0:1], vr)
    nc.vector.tensor_copy(gab[:, :, 1:2], pg[:, :, 0:1])
    return gab


def _stats_norm(g, bi, xt, gab):
    """Expand group stats to channels (ONE tiny PE matmul) + normalize."""
    nc = g.nc
    ps_ab = g.aux.tile([128, 4, 2], F32, tag="aux", name=f"ps_ab{bi}")
    nc.tensor.matmul(ps_ab, g.e8, gab, start=True, stop=True)
    # A = rstd * gn_scale ; Bb = gn_bias - mean * A
    ab = g.small.tile([128, 4, 2], F32, tag="ab", name=f"ab{bi}")
    tmpc = g.small.tile([128, 4, 1], F32, tag="tmpc", name=f"tmpc{bi}")
    nc.vector.tensor_mul(ab[:, :, 0:1], ps_ab[:, :, 0:1], g.gns_sb[:, :, 0:1])
    nc.vector.tensor_mul(tmpc, ps_ab[:, :, 1:2], ab[:, :, 0:1])
    nc.vector.tensor_sub(ab[:, :, 1:2], g.gnb_sb[:, :, 0:1], tmpc)
    zt = g.zt_p.tile([128, 4, 1024], SDT, tag="zt", name=f"zt{bi}")
    for t in range(4):
        if t % 2 == 0:
            nc.vector.tensor_scalar(
                out=zt[:, t, :], in0=xt[:, t, :],
                scalar1=ab[:, t, 0:1], scalar2=ab[:, t, 1:2],
                op0=ALU.mult, op1=ALU.add,
            )
        else:
            nc.scalar.activation(
                zt[:, t, :], xt[:, t, :], AF.Identity,
                scale=ab[:, t, 0:1], bias=ab[:, t, 1:2])
    return zt


def _gv_stage(g, bi, zt, tr=None):
    """G^T = M'^T Z^T and V' = Z Wv' (natural); interleave the next
    example's transpose groups (dependency-free PE work)."""
    nc = g.nc
    gt = g.gt_p.tile([128, 4, 1024], SDT, tag="gt")
    for m in range(4):
        ps = [g.pm.tile([128, 512], F32, tag="pm", name=f"ps_g{bi}_{m}_{h}")
              for h in range(2)]
        for kk in range(4):
            for h in range(2):
                nc.tensor.matmul(
                    ps[h],
                    g.mq_sb[:, kk, m * 128:(m + 1) * 128],
                    zt[:, kk, h * 512:(h + 1) * 512],
                    start=(kk == 0),
                    stop=(kk == 3),
                )
        for h in range(2):
            nc.scalar.copy(gt[:, m, h * 512:(h + 1) * 512], ps[h])
    v = g.v_p.tile([128, 8, 512], ODT, tag="v")
    for i in range(8):
        ps = g.pm.tile([128, 512], F32, tag="pm")
        for kk in range(4):
            nc.tensor.matmul(
                ps,
                zt[:, kk, i * 128:(i + 1) * 128],
                g.wvp_sb[:, kk, :],
                start=(kk == 0),
                stop=(kk == 3),
            )
        nc.scalar.copy(v[:, i, :], ps)
    return gt, v


def _u_stage(g, bi, zt):
    """Per-key bias u_j = uvec . z_j  (only when b_qkv != 0)."""
    nc = g.nc
    ps_u = g.aux.tile([128, 8], F32, tag="aux", name=f"ps_u{bi}")
    for j in range(8):
        for kk in range(4):
            nc.tensor.matmul(
                ps_u[:, j:j + 1],
                zt[:, kk, j * 128:(j + 1) * 128],
                g.uv_sb[:, kk:kk + 1],
                start=(kk == 0),
                stop=(kk == 3),
            )
    u_sb = g.small.tile([128, 8], F32, tag="u_sb", name=f"u_sb{bi}")
    nc.vector.tensor_scalar(out=u_sb, in0=ps_u, scalar1=1.0, scalar2=-2.0,
                            op0=ALU.mult, op1=ALU.add)
    return u_sb


def _phase_st(g, bi, zt, gt, u_sb=None, tr=None):
    """Transposed scores + exp -> ET; optionally interleave the next
    example's transpose groups (dependency-free PE work).  Softmax
    denominators accumulate via M=1 ones-lhsT DoubleRow matmuls (trivial
    weight load) into a [1, 1024] row as the ET chunks appear."""
    nc = g.nc
    et = g.et_p.tile([128, 8, 1024], ODT, tag="et")
    s_ps = [g.aux.tile([1, 512], F32, tag="aux", name=f"s_ps{bi}_{h}")
            for h in range(2)]
    for j in range(8):
        ps = [g.pm.tile([128, 512], F32, tag="pm", name=f"ps_s{bi}_{j}_{h}")
              for h in range(2)]
        for ct in range(4):
            for h in range(2):
                nc.tensor.matmul(
                    ps[h],
                    zt[:, ct, j * 128:(j + 1) * 128],
                    gt[:, ct, h * 512:(h + 1) * 512],
                    start=(ct == 0),
                    stop=(ct == 3),
                )
        for h in range(2):
            nc.scalar.activation(
                et[:, j, h * 512:(h + 1) * 512], ps[h], AF.Exp,
                scale=1.0 / SCALE_M,
                bias=g.neg2 if u_sb is None else u_sb[:, j:j + 1])
        if tr is not None:
            tr(j)
        if j % 2 == 1:
            jj = j // 2
            for h in range(2):
                nc.tensor.matmul(
                    s_ps[h],
                    g.ones2[:, :, 0:1],
                    et[:, 2 * jj:2 * jj + 2, h * 512:(h + 1) * 512],
                    start=(jj == 0),
                    stop=(jj == 3),
                    perf_mode=DR,
                )
    return et, s_ps


def _denom(g, bi, s_ps):
    """Denominator row -> column layout via a DRAM bounce; recip scale."""
    nc = g.nc
    s_sb = g.small.tile([1, 1024], F32, tag="s_sb", name=f"s_sb{bi}")
    for h in range(2):
        nc.vector.tensor_copy(s_sb[:, h * 512:(h + 1) * 512], s_ps[h])
    s_dram = g.dram.tile([1, 1024], F32, tag="s_dram", name=f"s_dram{bi}")
    nc.sync.dma_start(s_dram, s_sb)
    s_col = g.small.tile([128, 8], F32, tag="s_col", name=f"s_col{bi}")
    nc.sync.dma_start(s_col, s_dram.rearrange("o (t p) -> p (o t)", p=128))
    rc = g.small.tile([128, 8], F32, tag="rc", name=f"rc{bi}")
    nc.vector.reciprocal(rc, s_col)
    return rc


def _phase_o(g, bi, xn, v, et, rc, mid1=None, mid2=None):
    """O natural, residual, store.  mid1/mid2 emit the next example's
    stats matmuls at points where their DVE inputs are already complete
    (no PE-FIFO stall)."""
    nc = g.nc
    res = g.res_p.tile([128, 8, 512], F32, tag="res")
    out_q = [nc.sync, nc.scalar]
    for i in range(8):
        ps_o = g.pm.tile([128, 512], F32, tag="pm", name=f"ps_o{bi}_{i}")
        for jj in range(4):
            nc.tensor.matmul(
                ps_o,
                et[:, 2 * jj:2 * jj + 2, i * 128:(i + 1) * 128],
                v[:, 2 * jj:2 * jj + 2, :],
                start=(jj == 0),
                stop=(jj == 3),
                perf_mode=DR,
            )
        # b' pre-add into xn (after the transposes/stats read xn)
        nc.gpsimd.tensor_add(xn[:, i, :], xn[:, i, :], g.bres_bc)
        nc.vector.scalar_tensor_tensor(
            out=res[:, i, :], in0=ps_o, scalar=rc[:, i:i + 1],
            in1=xn[:, i, :], op0=ALU.mult, op1=ALU.add,
        )
        out_q[i % 2].dma_start(g.outr[bi, :, i, :], res[:, i, :])
        if i == 2 and mid1 is not None:
            mid1()
        if i == 4 and mid2 is not None:
            mid2()


def build_program(has_u):
    nc = bacc.Bacc("TRN2", target_bir_lowering=False, debug=False)

    x_d = nc.dram_tensor("x", [BPC, N, C], F32, kind="ExternalInput")
    mq_d = nc.dram_tensor("m_qk", [C, C], SDT, kind="ExternalInput")
    wvp_d = nc.dram_tensor("w_vp", [C, C], SDT, kind="ExternalInput")
    bres_d = nc.dram_tensor("b_res", [C], F32, kind="ExternalInput")
    gns_d = nc.dram_tensor("gn_scale", [C], F32, kind="ExternalInput")
    gnb_d = nc.dram_tensor("gn_bias", [C], F32, kind="ExternalInput")
    if has_u:
        uv_d = nc.dram_tensor("u_vec", [C], F32, kind="ExternalInput")
    out_d = nc.dram_tensor("out", [BPC, N, C], F32, kind="ExternalOutput")

    g = Ctx()
    g.nc = nc
    g.xr = x_d.ap().rearrange("b (i p) c -> b p i c", p=128)
    g.outr = out_d.ap().rearrange("b (i p) c -> b p i c", p=128)

    with tile.TileContext(nc) as tc:
        from contextlib import ExitStack
        with ExitStack() as ctx:
            const = ctx.enter_context(tc.tile_pool(name="const", bufs=1))
            g.pm = ctx.enter_context(tc.tile_pool(name="pm", bufs=6, space=MS.PSUM))
            g.aux = ctx.enter_context(tc.tile_pool(name="aux", bufs=2, space=MS.PSUM))
            g.xn_p = ctx.enter_context(tc.tile_pool(name="xn", bufs=2))
            g.xt_p = ctx.enter_context(tc.tile_pool(name="xtp", bufs=1))
            g.zt_p = ctx.enter_context(tc.tile_pool(name="ztp", bufs=2))
            g.gt_p = ctx.enter_context(tc.tile_pool(name="gtp", bufs=1))
            g.v_p = ctx.enter_context(tc.tile_pool(name="vp", bufs=2))
            g.et_p = ctx.enter_context(tc.tile_pool(name="etp", bufs=2))
            g.res_p = ctx.enter_context(tc.tile_pool(name="resp", bufs=1))
            g.small = ctx.enter_context(tc.tile_pool(name="small", bufs=3))
            g.dram = ctx.enter_context(tc.tile_pool(name="dram", bufs=2,
                                                    space=MS.DRAM))

            # ---- example-0 input DMA first: it is on the critical path
            xn0 = _load_x(g, 0, nq=3)

            # ---- constants ----------------------------------------------
            g.ident = const.tile([128, 128], F32)
            make_identity(nc, g.ident)

            # PE warmup: real matmuls with no DMA dependency, issued while
            # the input DMAs run, so the HAM clock gate reaches K=8/8
            # before the first productive matmul.
            def warm(n, salt=[0]):
                for _ in range(n):
                    salt[0] += 1
                    ps_w = g.pm.tile([128, 512], F32, tag="pm",
                                     name=f"ps_w{salt[0]}")
                    nc.tensor.matmul(ps_w[:, 0:128], g.ident, g.ident,
                                     start=True, stop=True)
            g.warm = warm
            warm(24)

            g.a_pool = const.tile([128, 8], F32)
            nc.gpsimd.memset(g.a_pool, 1.0 / GS)
            nc.gpsimd.affine_select(
                out=g.a_pool, in_=g.a_pool, compare_op=ALU.is_ge, fill=0.0,
                base=0, pattern=[[-GS, 8]], channel_multiplier=1)
            nc.gpsimd.affine_select(
                out=g.a_pool, in_=g.a_pool, compare_op=ALU.is_ge, fill=0.0,
                base=GS - 1, pattern=[[GS, 8]], channel_multiplier=-1)

            g.e8 = const.tile([8, 128], F32)
            nc.gpsimd.memset(g.e8, 1.0)
            nc.gpsimd.affine_select(
                out=g.e8, in_=g.e8, compare_op=ALU.is_ge, fill=0.0,
                base=0, pattern=[[1, 128]], channel_multiplier=-GS)
            nc.gpsimd.affine_select(
                out=g.e8, in_=g.e8, compare_op=ALU.is_ge, fill=0.0,
                base=GS - 1, pattern=[[-1, 128]], channel_multiplier=GS)

            g.ones2 = const.tile([128, 2, 16], ODT)
            nc.vector.memset(g.ones2, SCALE_V)
            g.neg2 = const.tile([128, 1], F32)
            nc.vector.memset(g.neg2, -2.0)
            g.eps_c = const.tile([128, 1], F32)
            nc.vector.memset(g.eps_c, EPS)

            g.mq_sb = const.tile([128, 4, C], SDT)
            mqr = mq_d.ap().rearrange("(t p) d -> t p d", p=128)
            for t in range(4):
                nc.gpsimd.dma_start(g.mq_sb[:, t, :], mqr[t])
            g.wvp_sb = const.tile([128, 4, C], SDT)
            wvr = wvp_d.ap().rearrange("(t p) d -> t p d", p=128)
            for t in range(4):
                nc.gpsimd.dma_start(g.wvp_sb[:, t, :], wvr[t])

            g.gns_sb = const.tile([128, 4, 1], F32)
            nc.sync.dma_start(g.gns_sb[:, :, 0:1],
                              gns_d.ap().rearrange("(t p) -> p t", p=128))
            g.gnb_sb = const.tile([128, 4, 1], F32)
            nc.sync.dma_start(g.gnb_sb[:, :, 0:1],
                              gnb_d.ap().rearrange("(t p) -> p t", p=128))

            def bcast(src_ap):
                return bass.AP(
                    tensor=src_ap.tensor, offset=src_ap.offset,
                    ap=[[0, 128]] + [list(p) for p in src_ap.ap])

            g.bres_bc = const.tile([128, 512], F32)
            nc.gpsimd.dma_start(g.bres_bc, bcast(bres_d.ap()))

            if has_u:
                uvf = const.tile([128, 4], F32)
                nc.sync.dma_start(uvf, uv_d.ap().rearrange("(t p) -> p t", p=128))
                g.uv_sb = const.tile([128, 4], SDT)
                nc.vector.tensor_copy(g.uv_sb, uvf)

            # ---- example-0 prologue -------------------------------------
            xt0 = g.xt_p.tile([128, 4, 1024], XT_DT, tag="xt", name="xt0")
            st6_0 = g.small.tile([128, 4, 2, 6], F32, tag="st6", name="st6_0")
            for t in range(4):
                for half in range(2):
                    _tr_group(g, 0, xn0, xt0, st6_0, t, half)
            warm(90)
            m2_0 = _stats_aggr(g, 0, st6_0)
            gab0 = _stats_pool(g, 0, m2_0)
            warm(16)
            zt0 = _stats_norm(g, 0, xt0, gab0)
            warm(16)

            # ---- pipelined per-example emission -------------------------
            state = (xn0, zt0)
            nxt = {}
            for bi in range(BPC):
                xn, zt = state
                tr = None
                if bi + 1 < BPC:
                    xn1 = _load_x(g, bi + 1)
                    xt1 = g.xt_p.tile([128, 4, 1024], XT_DT, tag="xt",
                                      name=f"xt{bi+1}")
                    st6 = g.small.tile([128, 4, 2, 6], F32, tag="st6",
                                       name=f"st6_{bi+1}")

                    def tr(j, xn1=xn1, xt1=xt1, st6=st6, b1=bi + 1):
                        _tr_group(g, b1, xn1, xt1, st6, j // 2, j % 2)
                gt, v = _gv_stage(g, bi, zt)
                u_sb = _u_stage(g, bi, zt) if has_u else None
                et, s_ps = _phase_st(g, bi, zt, gt, u_sb, tr)
                mid1 = mid2 = None
                if bi + 1 < BPC:
                    # aggr first in the DVE FIFO, ahead of the denominator
                    # bounce and the O-phase stt ops
                    nxt["m2"] = _stats_aggr(g, bi + 1, st6)
                rc = _denom(g, bi, s_ps)
                if bi + 1 < BPC:

                    def mid1(b1=bi + 1):
                        nxt["gab"] = _stats_pool(g, b1, nxt["m2"])

                    def mid2(xt1=xt1, b1=bi + 1):
                        nxt["zt"] = _stats_norm(g, b1, xt1, nxt["gab"])
                _phase_o(g, bi, xn, v, et, rc, mid1, mid2)
                if bi + 1 < BPC:
                    state = (xn1, nxt["zt"])

    nc.compile()
    return nc


_NC = {}


def _get_nc(has_u):
    if has_u not in _NC:
        _NC[has_u] = build_program(has_u)
    return _NC[has_u]


def kernel(x, t, gn_scale, gn_bias, w_qkv, b_qkv, w_out, b_out):
    import ml_dtypes
    x = np.ascontiguousarray(np.asarray(x, np.float32).reshape(B, N, C))
    w_qkv = np.asarray(w_qkv, np.float32)
    b_qkv = np.asarray(b_qkv, np.float32)
    w_out = np.asarray(w_out, np.float32)
    b_out = np.asarray(b_out, np.float32)
    wq, wk, wv = w_qkv[:, 0:C], w_qkv[:, C:2 * C], w_qkv[:, 2 * C:3 * C]
    bq, bv = b_qkv[0:C], b_qkv[2 * C:3 * C]

    m_qk = ((wq @ wk.T) * (ISQ * SCALE_M)).astype(ml_dtypes.bfloat16)
    w_vp = ((wv @ w_out) * SCALE_V).astype(ml_dtypes.bfloat16)
    b_res = (bv @ w_out + b_out).astype(np.float32)
    u_vec = ((wk @ bq) * ISQ).astype(np.float32)
    has_u = bool(np.any(u_vec != 0.0))

    shared = {
        "m_qk": np.ascontiguousarray(m_qk),
        "w_vp": np.ascontiguousarray(w_vp),
        "b_res": np.ascontiguousarray(b_res),
        "gn_scale": np.ascontiguousarray(np.asarray(gn_scale, np.float32)),
        "gn_bias": np.ascontiguousarray(np.asarray(gn_bias, np.float32)),
    }
    if has_u:
        shared["u_vec"] = np.ascontiguousarray(u_vec)
    in_maps = [
        {"x": x[c * BPC:(c + 1) * BPC], **shared} for c in range(NCORES)
    ]
    nc = _get_nc(has_u)
    res = run_bass_kernel_spmd(nc, in_maps, core_ids=list(range(NCORES)))
    out = np.concatenate([res.results[c]["out"] for c in range(NCORES)], axis=0)
    return out.reshape(B, H, W, C)


# revision 33
# speedup vs baseline: 1.1550x; 1.0083x over previous
"""TRN2 Bass kernel for nn_Attention_20444044329649.

GroupNorm(32) -> qkv dense -> single-head spatial attention (1024 pos) ->
out dense -> residual.  B=32 examples sharded 4-per-core across 8 cores;
params replicated.

v3 — v2's algebraic folds plus PE-FIFO discipline:

  * scores:  S*isq = Z M' Z^T with M' = isq*Wq Wk^T host-precomputed;
    device computes G^T = M'^T Z^T only (no K projection).  q/k biases:
    per-query term cancels in softmax; per-key term u_j applied as a
    per-partition exp bias (only emitted when b_qkv != 0).
  * out-proj fold: Wv' = Wv W_out, b' = bv W_out + b_out; O computed in
    NATURAL layout via lhsT=ET chunks, rhs=V'.
  * softmax denominators: N=1 matmuls interleaved with the O matmuls.
  * big matmul operands bf16; accumulation fp32 in PSUM; residual fp32.
  * GroupNorm stats batched to exactly TWO tiny PE matmuls per example
    (pool + expand across all 4 channel chunks at once), emitted inside
    the PREVIOUS example's O phase where their DVE inputs are already
    complete — the PE strict-FIFO queue never waits on the serial DVE
    stats chain.
  * next example's x transposes interleaved into the ST j-loop: no-dep
    PE work that keeps the HAM activity window busy (transpose-mode gaps
    plus boundary stalls previously re-throttled the PE to 1.2 GHz for
    ~10 us every example).
  * zt/v/et double-buffered so cross-example WAR hazards never
    serialize; example-0 input DMA spread over 4 queues.
"""

import numpy as np

import concourse.bass as bass
import concourse.mybir as mybir
import concourse.tile as tile
from concourse import bacc
from concourse.bass_utils import run_bass_kernel_spmd
from concourse.masks import make_identity

B, H, W, C = 32, 32, 32, 512
N = H * W                      # 1024 positions
G = 32                         # groups
GS = C // G                    # 16 channels per group
EPS = 1e-5
NCORES = 8
BPC = B // NCORES              # 4 examples per core
ISQ = float(1.0 / np.sqrt(C))  # score scale (folded into M' on host)

F32 = mybir.dt.float32
BF16 = mybir.dt.bfloat16
FP8 = mybir.dt.float8e4
AF = mybir.ActivationFunctionType
ALU = mybir.AluOpType
MS = bass.MemorySpace
DR = mybir.MatmulPerfMode.DoubleRow

SDT = BF16                     # score path (zt/gt/mq/wvp) dtype
ODT = FP8                      # attention-weight path (et/v) dtype
XT_DT = BF16                   # x^T / stats path stays bf16
SCALE_M = 512.0                # host upscale on M'; exp scale undoes it
SCALE_V = 1.0                  # v path scale (1.0: bf16 proj, fp8 storage)


class Ctx:
    pass


def _load_x(g, bi, nq=2):
    xn = g.xn_p.tile([128, 8, 512], F32, tag="xn", name=f"xn{bi}")
    qs = [g.nc.sync, g.nc.scalar, g.nc.gpsimd][:nq]
    for d in range(8):
        qs[d % nq].dma_start(xn[:, d, :], g.xr[bi, :, d, :])
    return xn


def _tr_group(g, bi, xn, xt, st6, t, half):
    """One transpose group: 4 PE transpose MMs -> PSUM -> xT copy -> stats."""
    nc = g.nc
    ps = g.pm.tile([128, 512], F32, tag="pm", name=f"ps_tr{bi}_{t}_{half}")
    for q in range(4):
        i = half * 4 + q
        nc.tensor.matmul(
            ps[:, q * 128:(q + 1) * 128],
            xn[:, i, t * 128:(t + 1) * 128],
            g.ident,
            is_transpose=True,
            start=(q == 0),
            stop=(q == 3),
        )
    nc.vector.tensor_copy(xt[:, t, half * 512:(half + 1) * 512], ps)
    nc.vector.bn_stats(st6[:, t, half, :], xt[:, t, half * 512:(half + 1) * 512])


def _stats_aggr(g, bi, st6):
    """DVE-only part: aggregate bn stats into m2 = [mean, E[x^2]]."""
    nc = g.nc
    mv = g.small.tile([128, 4, 2], F32, tag="mv", name=f"mv{bi}")
    for t in range(4):
        nc.vector.bn_aggr(mv[:, t, :], st6[:, t, :, :])
    m2 = g.small.tile([128, 4, 2], F32, tag="m2", name=f"m2{bi}")
    nc.vector.tensor_copy(m2[:, :, 0:1], mv[:, :, 0:1])
    nc.vector.tensor_mul(m2[:, :, 1:2], mv[:, :, 0:1], mv[:, :, 0:1])
    nc.vector.tensor_add(m2[:, :, 1:2], m2[:, :, 1:2], mv[:, :, 1:2])
    return m2


def _stats_pool(g, bi, m2):
    """Pool over the 16-channel groups (ONE tiny PE matmul) and produce
    per-group [rstd, mean] on 8 partitions."""
    nc = g.nc
    ps_g = g.aux.tile([8, 4, 2], F32, tag="aux", name=f"ps_g{bi}")
    nc.tensor.matmul(ps_g, g.a_pool, m2, start=True, stop=True)
    pg = g.small.tile([8, 4, 2], F32, tag="pg", name=f"pg{bi}")
    nc.vector.tensor_copy(pg, ps_g)
    vr = g.small.tile([8, 4, 1], F32, tag="vr", name=f"vr{bi}")
    nc.vector.tensor_mul(vr, pg[:, :, 0:1], pg[:, :, 0:1])
    nc.vector.tensor_sub(vr, pg[:, :, 1:2], vr)
    nc.scalar.activation(vr, vr, AF.Sqrt, bias=g.eps_c[:8])
    nc.vector.reciprocal(vr, vr)
    gab = g.small.tile([8, 4, 2], F32, tag="gab", name=f"gab{bi}")
    nc.vector.tensor_copy(gab[:, :, 0:1], vr)
    nc.vector.tensor_copy(gab[:, :, 1:2], pg[:, :, 0:1])
    return gab


def _stats_norm(g, bi, xt, gab):
    """Expand group stats to channels (ONE tiny PE matmul) + normalize."""
    nc = g.nc
    ps_ab = g.aux.tile([128, 4, 2], F32, tag="aux", name=f"ps_ab{bi}")
    nc.tensor.matmul(ps_ab, g.e8, gab, start=True, stop=True)
    # A = rstd * gn_scale ; Bb = gn_bias - mean * A
    ab = g.small.tile([128, 4, 2], F32, tag="ab", name=f"ab{bi}")
    tmpc = g.small.tile([128, 4, 1], F32, tag="tmpc", name=f"tmpc{bi}")
    nc.vector.tensor_mul(ab[:, :, 0:1], ps_ab[:, :, 0:1], g.gns_sb[:, :, 0:1])
    nc.vector.tensor_mul(tmpc, ps_ab[:, :, 1:2], ab[:, :, 0:1])
    nc.vector.tensor_sub(ab[:, :, 1:2], g.gnb_sb[:, :, 0:1], tmpc)
    zt = g.zt_p.tile([128, 4, 1024], SDT, tag="zt", name=f"zt{bi}")
    for t in range(4):
        if t % 2 == 0:
            nc.vector.tensor_scalar(
                out=zt[:, t, :], in0=xt[:, t, :],
                scalar1=ab[:, t, 0:1], scalar2=ab[:, t, 1:2],
                op0=ALU.mult, op1=ALU.add,
            )
        else:
            nc.scalar.activation(
                zt[:, t, :], xt[:, t, :], AF.Identity,
                scale=ab[:, t, 0:1], bias=ab[:, t, 1:2])
    return zt


def _gv_stage(g, bi, zt, tr=None):
    """G^T = M'^T Z^T and V' = Z Wv' (natural); interleave the next
    example's transpose groups (dependency-free PE work)."""
    nc = g.nc
    gt = g.gt_p.tile([128, 4, 1024], SDT, tag="gt")
    for m in range(4):
        ps = [g.pm.tile([128, 512], F32, tag="pm", name=f"ps_g{bi}_{m}_{h}")
              for h in range(2)]
        for kk in range(4):
            for h in range(2):
                nc.tensor.matmul(
                    ps[h],
                    g.mq_sb[:, kk, m * 128:(m + 1) * 128],
                    zt[:, kk, h * 512:(h + 1) * 512],
                    start=(kk == 0),
                    stop=(kk == 3),
                )
        for h in range(2):
            nc.scalar.copy(gt[:, m, h * 512:(h + 1) * 512], ps[h])
    v = g.v_p.tile([128, 8, 512], ODT, tag="v")
    for i in range(8):
        ps = g.pm.tile([128, 512], F32, tag="pm")
        for kk in range(4):
            nc.tensor.matmul(
                ps,
                zt[:, kk, i * 128:(i + 1) * 128],
                g.wvp_sb[:, kk, :],
                start=(kk == 0),
                stop=(kk == 3),
            )
        nc.scalar.copy(v[:, i, :], ps)
    return gt, v


def _u_stage(g, bi, zt):
    """Per-key bias u_j = uvec . z_j  (only when b_qkv != 0)."""
    nc = g.nc
    ps_u = g.aux.tile([128, 8], F32, tag="aux", name=f"ps_u{bi}")
    for j in range(8):
        for kk in range(4):
            nc.tensor.matmul(
                ps_u[:, j:j + 1],
                zt[:, kk, j * 128:(j + 1) * 128],
                g.uv_sb[:, kk:kk + 1],
                start=(kk == 0),
                stop=(kk == 3),
            )
    u_sb = g.small.tile([128, 8], F32, tag="u_sb", name=f"u_sb{bi}")
    nc.vector.tensor_scalar(out=u_sb, in0=ps_u, scalar1=1.0, scalar2=-2.0,
                            op0=ALU.mult, op1=ALU.add)
    return u_sb


def _phase_st(g, bi, zt, gt, u_sb=None, tr=None):
    """Transposed scores + exp -> ET; optionally interleave the next
    example's transpose groups (dependency-free PE work).  Softmax
    denominators accumulate via M=1 ones-lhsT DoubleRow matmuls (trivial
    weight load) into a [1, 1024] row as the ET chunks appear."""
    nc = g.nc
    et = g.et_p.tile([128, 8, 1024], ODT, tag="et")
    s_ps = [g.aux.tile([1, 512], F32, tag="aux", name=f"s_ps{bi}_{h}")
            for h in range(2)]
    for j in range(8):
        ps = [g.pm.tile([128, 512], F32, tag="pm", name=f"ps_s{bi}_{j}_{h}")
              for h in range(2)]
        for ct in range(4):
            for h in range(2):
                nc.tensor.matmul(
                    ps[h],
                    zt[:, ct, j * 128:(j + 1) * 128],
                    gt[:, ct, h * 512:(h + 1) * 512],
                    start=(ct == 0),
                    stop=(ct == 3),
                )
        for h in range(2):
            nc.scalar.activation(
                et[:, j, h * 512:(h + 1) * 512], ps[h], AF.Exp,
                scale=1.0 / SCALE_M,
                bias=g.neg2 if u_sb is None else u_sb[:, j:j + 1])
        if tr is not None:
            tr(j)
        if j % 2 == 1:
            jj = j // 2
            for h in range(2):
                nc.tensor.matmul(
                    s_ps[h],
                    g.ones2[:, :, 0:1],
                    et[:, 2 * jj:2 * jj + 2, h * 512:(h + 1) * 512],
                    start=(jj == 0),
                    stop=(jj == 3),
                    perf_mode=DR,
                )
    return et, s_ps


def _denom(g, bi, s_ps):
    """Denominator row -> column layout via a DRAM bounce; recip scale."""
    nc = g.nc
    s_sb = g.small.tile([1, 1024], F32, tag="s_sb", name=f"s_sb{bi}")
    for h in range(2):
        nc.vector.tensor_copy(s_sb[:, h * 512:(h + 1) * 512], s_ps[h])
    s_dram = g.dram.tile([1, 1024], F32, tag="s_dram", name=f"s_dram{bi}")
    nc.sync.dma_start(s_dram, s_sb)
    s_col = g.small.tile([128, 8], F32, tag="s_col", name=f"s_col{bi}")
    nc.sync.dma_start(s_col, s_dram.rearrange("o (t p) -> p (o t)", p=128))
    rc = g.small.tile([128, 8], F32, tag="rc", name=f"rc{bi}")
    nc.vector.reciprocal(rc, s_col)
    return rc


def _phase_o(g, bi, xn, v, et, rc, mid1=None, mid2=None):
    """O natural, residual, store.  mid1/mid2 emit the next example's
    stats matmuls at points where their DVE inputs are already complete
    (no PE-FIFO stall)."""
    nc = g.nc
    res = g.res_p.tile([128, 8, 512], F32, tag="res")
    out_q = [nc.sync, nc.scalar]
    for i in range(8):
        ps_o = g.pm.tile([128, 512], F32, tag="pm", name=f"ps_o{bi}_{i}")
        for jj in range(4):
            nc.tensor.matmul(
                ps_o,
                et[:, 2 * jj:2 * jj + 2, i * 128:(i + 1) * 128],
                v[:, 2 * jj:2 * jj + 2, :],
                start=(jj == 0),
                stop=(jj == 3),
                perf_mode=DR,
            )
        # b' pre-add into xn (after the transposes/stats read xn)
        nc.gpsimd.tensor_add(xn[:, i, :], xn[:, i, :], g.bres_bc)
        nc.vector.scalar_tensor_tensor(
            out=res[:, i, :], in0=ps_o, scalar=rc[:, i:i + 1],
            in1=xn[:, i, :], op0=ALU.mult, op1=ALU.add,
        )
        out_q[i % 2].dma_start(g.outr[bi, :, i, :], res[:, i, :])
        if i == 3 and mid1 is not None:
            mid1()
        if i == 5 and mid2 is not None:
            mid2()


def build_program(has_u):
    nc = bacc.Bacc("TRN2", target_bir_lowering=False, debug=False)

    x_d = nc.dram_tensor("x", [BPC, N, C], F32, kind="ExternalInput")
    mq_d = nc.dram_tensor("m_qk", [C, C], SDT, kind="ExternalInput")
    wvp_d = nc.dram_tensor("w_vp", [C, C], SDT, kind="ExternalInput")
    bres_d = nc.dram_tensor("b_res", [C], F32, kind="ExternalInput")
    gns_d = nc.dram_tensor("gn_scale", [C], F32, kind="ExternalInput")
    gnb_d = nc.dram_tensor("gn_bias", [C], F32, kind="ExternalInput")
    if has_u:
        uv_d = nc.dram_tensor("u_vec", [C], F32, kind="ExternalInput")
    out_d = nc.dram_tensor("out", [BPC, N, C], F32, kind="ExternalOutput")

    g = Ctx()
    g.nc = nc
    g.xr = x_d.ap().rearrange("b (i p) c -> b p i c", p=128)
    g.outr = out_d.ap().rearrange("b (i p) c -> b p i c", p=128)

    with tile.TileContext(nc) as tc:
        from contextlib import ExitStack
        with ExitStack() as ctx:
            const = ctx.enter_context(tc.tile_pool(name="const", bufs=1))
            g.pm = ctx.enter_context(tc.tile_pool(name="pm", bufs=6, space=MS.PSUM))
            g.aux = ctx.enter_context(tc.tile_pool(name="aux", bufs=2, space=MS.PSUM))
            g.xn_p = ctx.enter_context(tc.tile_pool(name="xn", bufs=2))
            g.xt_p = ctx.enter_context(tc.tile_pool(name="xtp", bufs=1))
            g.zt_p = ctx.enter_context(tc.tile_pool(name="ztp", bufs=2))
            g.gt_p = ctx.enter_context(tc.tile_pool(name="gtp", bufs=1))
            g.v_p = ctx.enter_context(tc.tile_pool(name="vp", bufs=2))
            g.et_p = ctx.enter_context(tc.tile_pool(name="etp", bufs=2))
            g.res_p = ctx.enter_context(tc.tile_pool(name="resp", bufs=1))
            g.small = ctx.enter_context(tc.tile_pool(name="small", bufs=3))
            g.dram = ctx.enter_context(tc.tile_pool(name="dram", bufs=2,
                                                    space=MS.DRAM))

            # ---- example-0 input DMA first: it is on the critical path
            xn0 = _load_x(g, 0, nq=3)

            # ---- constants ----------------------------------------------
            g.ident = const.tile([128, 128], F32)
            make_identity(nc, g.ident)

            # PE warmup: real matmuls with no DMA dependency, issued while
            # the input DMAs run, so the HAM clock gate reaches K=8/8
            # before the first productive matmul.
            def warm(n, salt=[0]):
                for _ in range(n):
                    salt[0] += 1
                    ps_w = g.pm.tile([128, 512], F32, tag="pm",
                                     name=f"ps_w{salt[0]}")
                    nc.tensor.matmul(ps_w[:, 0:128], g.ident, g.ident,
                                     start=True, stop=True)
            g.warm = warm
            warm(24)

            g.a_pool = const.tile([128, 8], F32)
            nc.gpsimd.memset(g.a_pool, 1.0 / GS)
            nc.gpsimd.affine_select(
                out=g.a_pool, in_=g.a_pool, compare_op=ALU.is_ge, fill=0.0,
                base=0, pattern=[[-GS, 8]], channel_multiplier=1)
            nc.gpsimd.affine_select(
                out=g.a_pool, in_=g.a_pool, compare_op=ALU.is_ge, fill=0.0,
                base=GS - 1, pattern=[[GS, 8]], channel_multiplier=-1)

            g.e8 = const.tile([8, 128], F32)
            nc.gpsimd.memset(g.e8, 1.0)
            nc.gpsimd.affine_select(
                out=g.e8, in_=g.e8, compare_op=ALU.is_ge, fill=0.0,
                base=0, pattern=[[1, 128]], channel_multiplier=-GS)
            nc.gpsimd.affine_select(
                out=g.e8, in_=g.e8, compare_op=ALU.is_ge, fill=0.0,
                base=GS - 1, pattern=[[-1, 128]], channel_multiplier=GS)

            g.ones2 = const.tile([128, 2, 16], ODT)
            nc.vector.memset(g.ones2, SCALE_V)
            g.neg2 = const.tile([128, 1], F32)
            nc.vector.memset(g.neg2, -2.0)
            g.eps_c = const.tile([128, 1], F32)
            nc.vector.memset(g.eps_c, EPS)

            g.mq_sb = const.tile([128, 4, C], SDT)
            mqr = mq_d.ap().rearrange("(t p) d -> t p d", p=128)
            for t in range(4):
                nc.gpsimd.dma_start(g.mq_sb[:, t, :], mqr[t])
            g.wvp_sb = const.tile([128, 4, C], SDT)
            wvr = wvp_d.ap().rearrange("(t p) d -> t p d", p=128)
            for t in range(4):
                nc.gpsimd.dma_start(g.wvp_sb[:, t, :], wvr[t])

            g.gns_sb = const.tile([128, 4, 1], F32)
            nc.sync.dma_start(g.gns_sb[:, :, 0:1],
                              gns_d.ap().rearrange("(t p) -> p t", p=128))
            g.gnb_sb = const.tile([128, 4, 1], F32)
            nc.sync.dma_start(g.gnb_sb[:, :, 0:1],
                              gnb_d.ap().rearrange("(t p) -> p t", p=128))

            def bcast(src_ap):
                return bass.AP(
                    tensor=src_ap.tensor, offset=src_ap.offset,
                    ap=[[0, 128]] + [list(p) for p in src_ap.ap])

            g.bres_bc = const.tile([128, 512], F32)
            nc.gpsimd.dma_start(g.bres_bc, bcast(bres_d.ap()))

            if has_u:
                uvf = const.tile([128, 4], F32)
                nc.sync.dma_start(uvf, uv_d.ap().rearrange("(t p) -> p t", p=128))
                g.uv_sb = const.tile([128, 4], SDT)
                nc.vector.tensor_copy(g.uv_sb, uvf)

            # ---- example-0 prologue -------------------------------------
            xt0 = g.xt_p.tile([128, 4, 1024], XT_DT, tag="xt", name="xt0")
            st6_0 = g.small.tile([128, 4, 2, 6], F32, tag="st6", name="st6_0")
            for t in range(4):
                for half in range(2):
                    _tr_group(g, 0, xn0, xt0, st6_0, t, half)
            warm(90)
            m2_0 = _stats_aggr(g, 0, st6_0)
            gab0 = _stats_pool(g, 0, m2_0)
            warm(16)
            zt0 = _stats_norm(g, 0, xt0, gab0)
            warm(16)

            # ---- pipelined per-example emission -------------------------
            state = (xn0, zt0)
            nxt = {}
            for bi in range(BPC):
                xn, zt = state
                tr = None
                if bi + 1 < BPC:
                    xn1 = _load_x(g, bi + 1)
                    xt1 = g.xt_p.tile([128, 4, 1024], XT_DT, tag="xt",
                                      name=f"xt{bi+1}")
                    st6 = g.small.tile([128, 4, 2, 6], F32, tag="st6",
                                       name=f"st6_{bi+1}")

                    def tr(j, xn1=xn1, xt1=xt1, st6=st6, b1=bi + 1):
                        _tr_group(g, b1, xn1, xt1, st6, j // 2, j % 2)
                gt, v = _gv_stage(g, bi, zt)
                u_sb = _u_stage(g, bi, zt) if has_u else None
                et, s_ps = _phase_st(g, bi, zt, gt, u_sb, tr)
                mid1 = mid2 = None
                if bi + 1 < BPC:
                    # aggr first in the DVE FIFO, ahead of the denominator
                    # bounce and the O-phase stt ops
                    nxt["m2"] = _stats_aggr(g, bi + 1, st6)
                rc = _denom(g, bi, s_ps)
                if bi + 1 < BPC:

                    def mid1(b1=bi + 1):
                        nxt["gab"] = _stats_pool(g, b1, nxt["m2"])

                    def mid2(xt1=xt1, b1=bi + 1):
                        nxt["zt"] = _stats_norm(g, b1, xt1, nxt["gab"])
                _phase_o(g, bi, xn, v, et, rc, mid1, mid2)
                if bi + 1 < BPC:
                    state = (xn1, nxt["zt"])

    nc.compile()
    return nc


_NC = {}


def _get_nc(has_u):
    if has_u not in _NC:
        _NC[has_u] = build_program(has_u)
    return _NC[has_u]


def kernel(x, t, gn_scale, gn_bias, w_qkv, b_qkv, w_out, b_out):
    import ml_dtypes
    x = np.ascontiguousarray(np.asarray(x, np.float32).reshape(B, N, C))
    w_qkv = np.asarray(w_qkv, np.float32)
    b_qkv = np.asarray(b_qkv, np.float32)
    w_out = np.asarray(w_out, np.float32)
    b_out = np.asarray(b_out, np.float32)
    wq, wk, wv = w_qkv[:, 0:C], w_qkv[:, C:2 * C], w_qkv[:, 2 * C:3 * C]
    bq, bv = b_qkv[0:C], b_qkv[2 * C:3 * C]

    m_qk = ((wq @ wk.T) * (ISQ * SCALE_M)).astype(ml_dtypes.bfloat16)
    w_vp = ((wv @ w_out) * SCALE_V).astype(ml_dtypes.bfloat16)
    b_res = (bv @ w_out + b_out).astype(np.float32)
    u_vec = ((wk @ bq) * ISQ).astype(np.float32)
    has_u = bool(np.any(u_vec != 0.0))

    shared = {
        "m_qk": np.ascontiguousarray(m_qk),
        "w_vp": np.ascontiguousarray(w_vp),
        "b_res": np.ascontiguousarray(b_res),
        "gn_scale": np.ascontiguousarray(np.asarray(gn_scale, np.float32)),
        "gn_bias": np.ascontiguousarray(np.asarray(gn_bias, np.float32)),
    }
    if has_u:
        shared["u_vec"] = np.ascontiguousarray(u_vec)
    in_maps = [
        {"x": x[c * BPC:(c + 1) * BPC], **shared} for c in range(NCORES)
    ]
    nc = _get_nc(has_u)
    res = run_bass_kernel_spmd(nc, in_maps, core_ids=list(range(NCORES)))
    out = np.concatenate([res.results[c]["out"] for c in range(NCORES)], axis=0)
    return out.reshape(B, H, W, C)
